# revision 20
# baseline (speedup 1.0000x reference)
"""CLUB loss kernel for Trainium2, 8 NeuronCores (SPMD data-parallel).

Math: with flat_x (N,d), iv = exp(-p_logvar):
  positive_i = -0.5 * sum_d (x_i - mu_i)^2 * iv_i
  negative_i = -0.5 * sum_d iv_i * (ex2 - 2 mu_i ex + mu_i^2)
  loss = mean_i(positive_i - negative_i)
Decomposed into global sums (single pass over data):
  sx[d], sxx[d], A[d]=sum iv, B2[d]=sum iv*mu, Ta=sum iv*x^2, Tb=sum iv*mu*x
  loss = -0.5/N * [(Ta - 2 Tb) - dot(sxx,A)/N + dot(sx,2*B2)/N]

v3 design (v2 measured 53.9us; trace breakdown: 8.8us fixed preamble to
first HBM byte, ~31us gapless DMA stream at line rate, ~9us compute tail
after the last byte, ~4.5us postamble):
 - The tail was the lever: v2's last tile was 2048 rows with lv arriving
   LAST, so exp->jj (GPSIMD, serial) gated the final M2 matmuls, the PE
   idled >3.4us and HAM re-throttled it to 1.2GHz right when it mattered.
 - v3 uses 9 groups (7x1024 + 2x512 rows). The last two groups DMA in
   mu,lv,x order so the x-chain (cast->transpose->square->matmul) is the
   only post-stream work, on a 512-row group: tail ~4us.
 - ~72 tiny warm-up matmuls run during the DMA-wait preamble (PE is idle
   there) so the HAM clock gate opens before real matmuls start, and the
   no-stall ordering keeps it open through the tail.
 - Wrap-up is spread across ACT (PSUM column copies) / DVE (diag
   extracts) / PE (partition fold) instead of a serial DVE chain.
Everything else keeps the v2 structure: contiguous DRAM loads (>=2KB per
partition chunk), i-major permuted mu/lv (partition p of group g holds
rows R*p..R*p+R-1), x PE-transposed per 128-col group with column stride
R reproducing the same permutation, coupled sums as PE matmuls with fp16
operands accumulated in fp32 PSUM over all 64 blocks:
  M1: lhsT=iv_blk,  rhs=[xsqT_blk | ones] -> diag = Ta partials, col128 = A
  M2: lhsT=jj_blk,  rhs=[xT_blk   | ones] -> diag = Tb partials, col128 = B2
  M3: lhsT=ones,    rhs=[xsqT_blk | ones] -> row = sxx partials
  M4: lhsT=ones,    rhs=[xT_blk   | ones] -> row = sx partials
Device emits per-core (128,4) + (2,128) stats; host does the O(d) combine.
"""

import numpy as np

B, D, H, W = 16, 128, 64, 64
N = B * H * W            # 65536
NCORES = 8
BPC = B // NCORES        # 2 batches per core
HW = H * W               # 4096
ROWS = BPC * HW          # 8192 rows per core
GROUPS = [1024] * 7 + [512] * 2   # rows per pipeline group (sum = ROWS)
NG = len(GROUPS)
GSTART = [sum(GROUPS[:i]) for i in range(NG)]
GBLK = [sum(GROUPS[:i]) // 128 for i in range(NG)]  # first global block
NBLK = ROWS // 128       # 64 blocks per core

_CACHE = {}


def _build_nc(stats_output=True):
    import concourse.bass as bass
    import concourse.bacc as bacc
    import concourse.mybir as mybir
    from concourse import masks
    from concourse.tile import TileContext

    f32 = mybir.dt.float32
    f16 = mybir.dt.float16
    ALU = mybir.AluOpType
    AF = mybir.ActivationFunctionType
    AX = mybir.AxisListType

    nc = bacc.Bacc(num_devices=NCORES)
    x_in = nc.dram_tensor("x", [BPC, D, HW], f32, kind="ExternalInput")
    mu_in = nc.dram_tensor("p_mu", [ROWS, D], f32, kind="ExternalInput")
    lv_in = nc.dram_tensor("p_logvar", [ROWS, D], f32, kind="ExternalInput")
    stats_out = nc.dram_tensor("stats", [128, 4], f32, kind="ExternalOutput")
    rows_out = nc.dram_tensor("rows", [1, 256], f32, kind="ExternalOutput")

    with TileContext(nc) as tc:
        with (
            tc.tile_pool(name="const", bufs=1) as constp,
            tc.tile_pool(name="slabs", bufs=NG) as slabs,
            tc.tile_pool(name="big", bufs=1) as big,
            tc.tile_pool(name="work", bufs=5) as work,
            tc.tile_pool(name="stats", bufs=1) as stats,
            tc.tile_pool(name="ps", bufs=4, space="PSUM") as psp,
            tc.tile_pool(name="psacc", bufs=1, space="PSUM") as psacc,
        ):
            # DMA plan. Per-transfer FIFO overhead on the HWDGE ring is
            # ~0.3us, so mu/lv ship as a few MULTI-GROUP transfers
            # (1-1.5MB, per-partition chunks 2-4KB): the rearrange
            # "(g p r) d -> p (g r d)" reproduces each group's row
            # permutation inside one transfer. x stays per-group so the
            # cast->transpose pipeline has fine-grained completions.
            # Chunk boundaries: A=g0-2, B=g3-5, C1=g6, C2=g7-8.
            CHUNKS = [(0, 3), (3, 3), (6, 1), (7, 2)]   # (first group, n)
            x_tiles = []
            for g, rows in enumerate(GROUPS):
                x_tiles.append(slabs.tile([128, 1024], f32, tag="x_t",
                                          name="x_t"))
            mu_tiles, lv_tiles = [], []
            for (g0, ng) in CHUNKS:
                crows = sum(GROUPS[g0:g0 + ng])
                mu_tiles.append(slabs.tile([128, crows], f32,
                                           tag=f"mu_c{g0}", name="mu_c",
                                           bufs=1))
                lv_tiles.append(slabs.tile([128, crows], f32,
                                           tag=f"lv_c{g0}", name="lv_c",
                                           bufs=1))

            def mu_lv_slice(g):
                """(mu_view, lv_view) [128, GROUPS[g]] for group g."""
                for ci, (g0, ng) in enumerate(CHUNKS):
                    if g0 <= g < g0 + ng:
                        off = sum(GROUPS[g0:g])
                        sl = slice(off, off + GROUPS[g])
                        return mu_tiles[ci][:, sl], lv_tiles[ci][:, sl]

            def dma_x(g):
                r0 = GSTART[g]
                b, hw0 = r0 // HW, r0 % HW
                rows = GROUPS[g]
                nc.sync.dma_start(out=x_tiles[g][:, :rows],
                                  in_=x_in[b, :, hw0:hw0 + rows])

            def dma_chunk(ci, src, tiles):
                g0, ng = CHUNKS[ci]
                r0, crows = GSTART[g0], sum(GROUPS[g0:g0 + ng])
                R = GROUPS[g0] // 128   # uniform within a chunk
                nc.sync.dma_start(
                    out=tiles[ci][:].rearrange("p (g r d) -> p g r d",
                                               g=ng, r=R),
                    in_=src[r0:r0 + crows, :].rearrange(
                        "(g p r) d -> p g r d", g=ng, p=128))

            # stream order: x0 first (longest chain), then mu/lv chunks
            # early enough that exp+jj complete before their groups' x
            # arrives; x5..x8 stream last so the tail is a short x-chain
            dma_x(0)
            dma_chunk(0, mu_in, mu_tiles); dma_chunk(0, lv_in, lv_tiles)
            dma_x(1); dma_x(2)
            dma_chunk(1, mu_in, mu_tiles); dma_chunk(1, lv_in, lv_tiles)
            dma_x(3); dma_x(4)
            dma_chunk(2, mu_in, mu_tiles); dma_chunk(2, lv_in, lv_tiles)
            dma_chunk(3, mu_in, mu_tiles); dma_chunk(3, lv_in, lv_tiles)
            dma_x(5); dma_x(6); dma_x(7); dma_x(8)

            ident16 = constp.tile([128, 128], f16, name="ident16")
            masks.make_identity(nc, ident16[:])
            identf = constp.tile([128, 128], f32, name="identf")
            masks.make_identity(nc, identf[:])
            onecell = constp.tile([1, 1], f32, name="onecell")
            nc.vector.memset(onecell[:], 1.0)
            ones_col = constp.tile([128, 1], f16, name="ones_col")
            nc.vector.memset(ones_col[:], 1.0)

            # PSUM accumulators for the coupled matmul streams
            P1 = psacc.tile([128, 129], f32, name="P1")
            P2 = psacc.tile([128, 129], f32, name="P2")
            P34 = psacc.tile([1, 258], f32, name="P34")

            # HAM warm-up: ~72 tiny matmuls during the DMA-wait preamble
            # (PE idle 7-13us). ~3.6us of sustained PE activity opens the
            # clock gate (1.2 -> 2.4 GHz) before the real stream starts;
            # the dense stream + no-stall tail then keep it open. Results
            # land in P3, which the first real M3 (start=True) clears.
            for _ in range(100):
                nc.tensor.matmul(P34[0:1, 0:32], ones_col[:],
                                 ident16[:, 0:32], start=True, stop=True,
                                 skip_group_check=True)

            # persistent transposed-x layout, interleaved per block:
            # [xsqT(128) | ones | xT(128) | ones] = 258 cols. M1 reads
            # cols 0:129, M2 reads 129:258, and ONE ones-matmul (M34,
            # N=258) covers both sxx and sx — merging the two ones
            # streams halves their PE slot cost (LDW-bound ~107ns each).
            comb = big.tile([128, NBLK * 258], f16, name="comb")
            comb_v = comb[:].rearrange("p (n c) -> p n c", c=258)
            nc.vector.memset(comb_v[:, :, 128:129], 1.0)
            nc.vector.memset(comb_v[:, :, 257:258], 1.0)

            ivs, jjs = {}, {}

            def emit_mm(g):
                """M1/M2/M3/M4 matmuls for group g's blocks.

                For the last group, all M1s come first so P1 completes
                as early as possible and the DVE diag-extract overlaps
                the remaining M2/M3/M4 matmuls.
                """
                iv, jj = ivs[g], jjs[g]
                R = GROUPS[g] // 128
                def m1(r):
                    blk = GBLK[g] + r
                    nc.tensor.matmul(
                        P1[:], iv[:, r * 128:(r + 1) * 128],
                        comb_v[:, blk, 0:129], start=blk == 0,
                        stop=blk == NBLK - 1, skip_group_check=True)
                def m2(r):
                    blk = GBLK[g] + r
                    nc.tensor.matmul(
                        P2[:], jj[:, r * 128:(r + 1) * 128],
                        comb_v[:, blk, 129:258], start=blk == 0,
                        stop=blk == NBLK - 1, skip_group_check=True)
                def m34(r):
                    blk = GBLK[g] + r
                    nc.tensor.matmul(
                        P34[:], ones_col[:],
                        comb_v[:, blk, 0:258], start=blk == 0,
                        stop=blk == NBLK - 1, skip_group_check=True)
                if g == NG - 1:
                    for r in range(R):
                        m1(r)
                    for r in range(R):
                        m2(r)
                    for r in range(R):
                        m34(r)
                else:
                    for r in range(R):
                        m1(r); m2(r); m34(r)

            # bufs = NG: every iv/jj fully resident — zero ring reuse, so
            # no WAR coupling between late exps and earlier consumers
            for g in range(NG):
                ivs[g] = work.tile([128, 1024], f16, tag="iv", name="iv",
                                   bufs=NG)
                jjs[g] = work.tile([128, 1024], f16, tag="jj", name="jj",
                                   bufs=NG)

            def emit_exp_jj(g):
                iv, jj = ivs[g], jjs[g]
                mu_t, lv_t = mu_lv_slice(g)
                for c0 in range(0, GROUPS[g], 512):
                    sl = slice(c0, c0 + 512)
                    # ACT: iv = exp(-lv) (f32 -> f16), 512-col chunks
                    # so jj can chase the exp
                    nc.scalar.activation(iv[:, sl], lv_t[:, sl],
                                         AF.Exp, bias=0.0, scale=-1.0)
                    # GPSIMD: j = iv * mu (mixed f16*f32 -> f16; the
                    # DVE mixed path is a microcode disaster)
                    nc.gpsimd.tensor_tensor(jj[:, sl], iv[:, sl],
                                            mu_t[:, sl], ALU.mult)

            def emit_xside(g):
                """cast -> transposes -> copy/square for group g."""
                rows = GROUPS[g]
                R = rows // 128
                x_t = x_tiles[g]
                xb = work.tile([128, 1024], f16, tag="xb", name="xb",
                               bufs=3)
                # ACT: xb = fp16(x); the transposes read stride-R
                # columns so they need the whole group cast
                nc.scalar.activation(xb[:, :rows], x_t[:, :rows],
                                     AF.Copy)
                # stride-R column view: xb_g[:, k, r] = xb[:, R*k + r],
                # so transpose block r puts row r0 + R*k + r on partition
                # k — exactly the mu/lv DMA permutation.
                xb_g = xb[:, :rows].rearrange("p (k s) -> p k s", s=R)
                psx = psp.tile([128, 1024], f16, tag="psx", name="psx")
                for r in range(R):
                    nc.tensor.transpose(psx[:, r * 128:(r + 1) * 128],
                                        xb_g[:, :, r], ident16[:])
                blk0 = GBLK[g]
                # DVE: plain copy into the ones-strided layout
                nc.vector.tensor_copy(
                    comb_v[:, blk0:blk0 + R, 129:257], psx[:, :R * 128])
                # DVE: square as xT(SBUF) * psx(PSUM) — keeps squares
                # off ACT and uses one read port per space
                nc.vector.tensor_tensor(
                    comb_v[:, blk0:blk0 + R, 0:128],
                    comb_v[:, blk0:blk0 + R, 129:257], psx[:, :R * 128],
                    ALU.mult)

            # program order mirrors arrival order: x0, then chunk A's
            # exps/jj, then bulk x-sides with a one-group matmul lag,
            # with later chunks' exp/jj interleaved at their arrival
            # points; the final x-chains and matmuls close it out
            emit_xside(0)
            emit_exp_jj(0); emit_exp_jj(1); emit_exp_jj(2)
            emit_xside(1); emit_mm(0)
            emit_xside(2); emit_mm(1)
            emit_exp_jj(3); emit_exp_jj(4); emit_exp_jj(5)
            emit_xside(3); emit_mm(2)
            emit_xside(4); emit_mm(3)
            emit_exp_jj(6); emit_exp_jj(7); emit_exp_jj(8)
            emit_xside(5); emit_mm(4)
            emit_xside(6); emit_mm(5)
            emit_xside(7); emit_mm(6)
            emit_xside(8); emit_mm(7)
            emit_mm(8)

            # ---- wrap-up: fold into gstat[128,4] + rows[2,128], spread
            # over ACT (PSUM column/row copies) and DVE (diag extracts)
            # so the serial chain after the last matmul stays short ----
            gstat = stats.tile([128, 4], f32, name="gstat")
            scratch = stats.tile([128, 128], f32, name="scratch")
            # A, B2 from the ones columns (ACT sits next to PSUM)
            nc.scalar.activation(gstat[:, 0:1], P1[:, 128:129], AF.Copy)
            nc.scalar.activation(gstat[:, 1:2], P2[:, 128:129], AF.Copy)
            # Ta, Tb from the diagonals (DVE)
            nc.vector.tensor_tensor(scratch[:], P1[:, 0:128], identf[:],
                                    ALU.mult)
            nc.vector.tensor_reduce(gstat[:, 2:3], scratch[:], axis=AX.X,
                                    op=ALU.add)
            nc.vector.tensor_tensor(scratch[:], P2[:, 0:128], identf[:],
                                    ALU.mult)
            nc.vector.tensor_reduce(gstat[:, 3:4], scratch[:], axis=AX.X,
                                    op=ALU.add)
            # sxx (P34 cols 0:128) and sx (P34 cols 129:257) rows go
            # out as-is; host reads them
            srow = stats.tile([1, 256], f32, name="srow")
            nc.scalar.activation(srow[0:1, 0:128], P34[0:1, 0:128],
                                 AF.Copy)
            nc.scalar.activation(srow[0:1, 128:256], P34[0:1, 129:257],
                                 AF.Copy)

            nc.sync.dma_start(out=rows_out[:], in_=srow[:])
            nc.sync.dma_start(out=stats_out[:], in_=gstat[:])

    return nc


MODE = "host"


def get_nc(use_collective=True, stats_output=True):
    key = ("nc_v9",)
    if key not in _CACHE:
        nc = _build_nc()
        if not nc.is_finalized():
            nc.finalize()
        _CACHE[key] = nc
    return _CACHE[key]


def make_in_maps(x, p_mu, p_logvar):
    x = np.ascontiguousarray(np.asarray(x, dtype=np.float32))
    p_mu = np.ascontiguousarray(np.asarray(p_mu, dtype=np.float32))
    p_logvar = np.ascontiguousarray(np.asarray(p_logvar, dtype=np.float32))
    in_maps = []
    for c in range(NCORES):
        in_maps.append({
            "x": np.ascontiguousarray(
                x[c * BPC:(c + 1) * BPC].reshape(BPC, D, HW)),
            "p_mu": np.ascontiguousarray(p_mu[c * ROWS:(c + 1) * ROWS]),
            "p_logvar": np.ascontiguousarray(
                p_logvar[c * ROWS:(c + 1) * ROWS]),
        })
    return in_maps


def kernel(x, p_mu, p_logvar):
    from concourse.bass_utils import run_bass_kernel_spmd

    in_maps = make_in_maps(x, p_mu, p_logvar)
    nc = get_nc()
    res = run_bass_kernel_spmd(nc, in_maps, list(range(NCORES)))
    s = np.zeros((128, 4), dtype=np.float64)
    rr = np.zeros((2, 128), dtype=np.float64)
    for c in range(NCORES):
        s += np.asarray(res.results[c]["stats"], dtype=np.float64)
        rr += np.asarray(res.results[c]["rows"],
                         dtype=np.float64).reshape(2, 128)
    A, B2p, Ta, Tb = (s[:, k] for k in range(4))
    sxx, sx = rr[0], rr[1]
    T = Ta.sum() - 2.0 * Tb.sum()
    loss = -0.5 / N * (T - sxx.dot(A) / N + sx.dot(2.0 * B2p) / N)
    return np.asarray(loss, dtype=np.float32).reshape(())


# revision 21
# speedup vs baseline: 1.0647x; 1.0647x over previous
"""CLUB loss kernel for Trainium2, 8 NeuronCores (SPMD data-parallel).

Math: with flat_x (N,d), iv = exp(-p_logvar):
  positive_i = -0.5 * sum_d (x_i - mu_i)^2 * iv_i
  negative_i = -0.5 * sum_d iv_i * (ex2 - 2 mu_i ex + mu_i^2)
  loss = mean_i(positive_i - negative_i)
Decomposed into global sums (single pass over data):
  sx[d], sxx[d], A[d]=sum iv, B2[d]=sum iv*mu, Ta=sum iv*x^2, Tb=sum iv*mu*x
  loss = -0.5/N * [(Ta - 2 Tb) - dot(sxx,A)/N + dot(sx,2*B2)/N]

v10 = v2 structure + tail surgery. Trace-informed invariants (measured):
 - 12 x 1MB transfers on one HWDGE ring stream gaplessly at HBM line
   rate (~31us); EVERY extra transfer costs ~0.1-0.3us and sub-0.5MB
   tails crawl, so do NOT fine-grain the DMA (v3-v9 all lost there).
 - The PE is LDWEIGHTS-bound at ~107ns per matmul slot; 64 transposes +
   192 matmuls (~27us) just fit under the DMA window. A 4th matmul
   stream does NOT fit (v4's M4 regression).
 - v2's tail (9us): lv3 landed LAST, so exp3->jj3 (GPSIMD, serial
   ~1.2us/chunk) gated the last M2s after the stream ended.
v10 fixes exactly that:
 - Tile 3 DMAs as mu,lv,x: exp3+jj3 run during x3's 1MB stream; the
   post-stream work is only cast3 -> transposes -> copy/square -> M(3)
   -> wrap-up.
 - ACT program order for tile 3 is exp-then-cast to match arrival.
 - Wrap-up: sxx row (P3) goes out via a second tiny DMA instead of the
   fp32 PE fold + DVE copies; A/B2 column copies moved to ACT; the last
   tile emits all M1s first so the P1 diag extract overlaps M2/M3.
Everything else is v2 verbatim: i-major permuted mu/lv (8KB contiguous
per partition), x PE-transposed per 128-col group with column stride 16,
coupled sums as fp16 matmuls accumulated in fp32 PSUM over 64 blocks:
  M1: lhsT=iv_blk,  rhs=[xsqT_blk | ones] -> diag = Ta partials, col128 = A
  M2: lhsT=jj_blk,  rhs=[xT_blk   | ones] -> diag = Tb partials, col128 = B2
  M3: lhsT=ones,    rhs=[xsqT_blk | ones] -> row = sxx partials
sx via DVE free-dim reduce of natural d-major x (engine balancing).
Device emits per-core (128,5) + (1,128) stats; host does the O(d) combine.
"""

import numpy as np

B, D, H, W = 16, 128, 64, 64
N = B * H * W            # 65536
NCORES = 8
BPC = B // NCORES        # 2 batches per core
HW = H * W               # 4096
ROWS = BPC * HW          # 8192 rows per core
TILE = 2048              # rows per tile
NT = ROWS // TILE        # 4 tiles per core
BLK = TILE // 128        # 16 transpose blocks per tile
NBLK = ROWS // 128       # 64 blocks per core

_CACHE = {}


def _build_nc(stats_output=True):
    import concourse.bass as bass
    import concourse.bacc as bacc
    import concourse.mybir as mybir
    from concourse import masks
    from concourse.tile import TileContext

    f32 = mybir.dt.float32
    f16 = mybir.dt.float16
    ALU = mybir.AluOpType
    AF = mybir.ActivationFunctionType
    AX = mybir.AxisListType

    nc = bacc.Bacc(num_devices=NCORES)
    x_in = nc.dram_tensor("x", [BPC, D, HW], f32, kind="ExternalInput")
    mu_in = nc.dram_tensor("p_mu", [ROWS, D], f32, kind="ExternalInput")
    lv_in = nc.dram_tensor("p_logvar", [ROWS, D], f32, kind="ExternalInput")
    stats_out = nc.dram_tensor("stats", [128, 5], f32, kind="ExternalOutput")
    rows_out = nc.dram_tensor("rows", [1, 128], f32, kind="ExternalOutput")

    with TileContext(nc) as tc:
        with (
            tc.tile_pool(name="const", bufs=1) as constp,
            tc.tile_pool(name="slabs", bufs=4) as slabs,
            tc.tile_pool(name="big", bufs=1) as big,
            tc.tile_pool(name="work", bufs=2) as work,
            tc.tile_pool(name="stats", bufs=1) as stats,
            tc.tile_pool(name="ps", bufs=2, space="PSUM") as psp,
            tc.tile_pool(name="psacc", bufs=1, space="PSUM") as psacc,
        ):
            # issue every input DMA first: the triggers have no deps, and
            # the 16 DMA engines stream ~12.6MB for ~31us — the earlier
            # they start, the earlier the whole pipeline finishes. The
            # LAST tile goes mu,lv,x so exp+jj finish during x3's
            # stream and only the short x-chain trails the last byte.
            slabs_xml = []
            for t in range(NT):
                b, h = t // 2, t % 2
                x_t = slabs.tile([128, TILE], f32, tag="x_t", name="x_t")
                mu_t = slabs.tile([128, TILE], f32, tag="mu_t", name="mu_t")
                lv_t = slabs.tile([128, TILE], f32, tag="lv_t", name="lv_t")
                r0 = t * TILE
                def dx(x_t=x_t, b=b, h=h):
                    nc.sync.dma_start(out=x_t[:],
                                      in_=x_in[b, :, h * TILE:(h + 1) * TILE])
                def dm(mu_t=mu_t, r0=r0):
                    nc.sync.dma_start(
                        out=mu_t[:],
                        in_=mu_in[r0:r0 + TILE, :].rearrange(
                            "(p r) d -> p (r d)", p=128))
                def dl(lv_t=lv_t, r0=r0):
                    nc.sync.dma_start(
                        out=lv_t[:],
                        in_=lv_in[r0:r0 + TILE, :].rearrange(
                            "(p r) d -> p (r d)", p=128))
                if t < NT - 1:
                    dx(); dm(); dl()
                else:
                    dm(); dl(); dx()
                slabs_xml.append((x_t, mu_t, lv_t))

            ident16 = constp.tile([128, 128], f16, name="ident16")
            masks.make_identity(nc, ident16[:])
            identf = constp.tile([128, 128], f32, name="identf")
            masks.make_identity(nc, identf[:])
            ones_col = constp.tile([128, 1], f16, name="ones_col")
            nc.vector.memset(ones_col[:], 1.0)

            # persistent transposed-x layouts with a ones column every 129
            xTs = big.tile([128, NBLK * 129], f16, name="xTs")
            xsqTs = big.tile([128, NBLK * 129], f16, name="xsqTs")
            xTs_v = xTs[:].rearrange("p (n c) -> p n c", c=129)
            xsqTs_v = xsqTs[:].rearrange("p (n c) -> p n c", c=129)
            nc.vector.memset(xTs_v[:, :, 128:129], 1.0)
            nc.vector.memset(xsqTs_v[:, :, 128:129], 1.0)

            # PSUM accumulators for the coupled matmul streams
            P1 = psacc.tile([128, 129], f32, name="P1")
            P2 = psacc.tile([128, 129], f32, name="P2")
            P3 = psacc.tile([1, 129], f32, name="P3")

            sx_cols = stats.tile([128, 2 * NT], f32, name="sx_cols")

            # per-tile state kept across the 1-half-tile software pipeline lag
            ivs, jjs = {}, {}

            def emit_mm(u, last_tile=False):
                """M1/M2/M3 matmuls for half-tile u (blocks 8u..8u+8).

                For the final half-tile, all M1s run first so P1 is done
                early and the DVE diag extract overlaps the M2/M3 tail.
                """
                t = u // 2
                iv, jj = ivs[t], jjs[t]
                def m1(k):
                    blk = u * 8 + k
                    r = blk % BLK
                    nc.tensor.matmul(
                        P1[:], iv[:, r * 128:(r + 1) * 128],
                        xsqTs_v[:, blk, :], start=blk == 0,
                        stop=blk == NBLK - 1, skip_group_check=True)
                def m2(k):
                    blk = u * 8 + k
                    r = blk % BLK
                    nc.tensor.matmul(
                        P2[:], jj[:, r * 128:(r + 1) * 128],
                        xTs_v[:, blk, :], start=blk == 0,
                        stop=blk == NBLK - 1, skip_group_check=True)
                def m3(k):
                    blk = u * 8 + k
                    nc.tensor.matmul(
                        P3[:], ones_col[:],
                        xsqTs_v[:, blk, :], start=blk == 0,
                        stop=blk == NBLK - 1, skip_group_check=True)
                if last_tile:
                    for k in range(8):
                        m1(k)
                    for k in range(8):
                        m2(k); m3(k)
                else:
                    for k in range(8):
                        m1(k); m2(k); m3(k)

            for t in range(NT):
                x_t, mu_t, lv_t = slabs_xml[t]
                iv = work.tile([128, TILE], f16, tag="iv", name="iv", bufs=4)
                jj = work.tile([128, TILE], f16, tag="jj", name="jj", bufs=4)
                xb = work.tile([128, TILE], f16, tag="xb", name="xb", bufs=4)
                ivs[t], jjs[t] = iv, jj

                def emit_cast(xb=xb, x_t=x_t):
                    # ACT: xb = fp16(x)  (the transposes read stride-16
                    # columns so they need all of xb)
                    nc.scalar.activation(xb[:], x_t[:], AF.Copy)

                def emit_exp_jj(iv=iv, jj=jj, mu_t=mu_t, lv_t=lv_t):
                    QT = TILE // 4
                    for q in range(4):
                        sl = slice(q * QT, (q + 1) * QT)
                        # ACT: iv = exp(-lv)  (f32 -> f16); quarters let
                        # jj chase the exp.
                        nc.scalar.activation(iv[:, sl], lv_t[:, sl], AF.Exp,
                                             bias=0.0, scale=-1.0)
                        # GPSIMD: j = iv * mu (mixed f16*f32 -> f16; DVE's
                        # mixed path is a 131 cyc/elem microcode disaster)
                        nc.gpsimd.tensor_tensor(jj[:, sl], iv[:, sl],
                                                mu_t[:, sl], ALU.mult)

                # ACT program order matches DMA arrival order: x first
                # for tiles 0-2, lv first for the last tile
                if t < NT - 1:
                    emit_cast(); emit_exp_jj()
                else:
                    emit_exp_jj(); emit_cast()

                HT = TILE // 2
                for hh in range(2):
                    sl = slice(hh * HT, (hh + 1) * HT)
                    # DVE: sx partial (d-major, f32)
                    nc.vector.tensor_reduce(sx_cols[:, 2 * t + hh:
                                                    2 * t + hh + 1],
                                            x_t[:, sl], axis=AX.X, op=ALU.add)

                # stride-16 column view of xb: xb_g[:, k, r] = xb[:, 16k+r]
                xb_g = xb[:].rearrange("p (k s) -> p k s", s=16)
                for hh in range(2):
                    u = t * 2 + hh
                    psx = psp.tile([128, 1024], f16, tag="psx", name="psx",
                                   bufs=4)
                    for k in range(8):
                        r = hh * 8 + k
                        nc.tensor.transpose(psx[:, k * 128:(k + 1) * 128],
                                            xb_g[:, :, r], ident16[:])
                    # DVE: plain copy into the ones-strided layout
                    blk0 = u * 8
                    nc.vector.tensor_copy(
                        xTs_v[:, blk0:blk0 + 8, 0:128], psx[:])
                    # DVE: square as xT(SBUF) * psx(PSUM) — keeps squares
                    # off ACT (so exp never queues behind them) and uses
                    # one read port per space (dual-PSUM TT is illegal)
                    nc.vector.tensor_tensor(
                        xsqTs_v[:, blk0:blk0 + 8, 0:128],
                        xTs_v[:, blk0:blk0 + 8, 0:128], psx[:], ALU.mult)
                    if u >= 2:
                        emit_mm(u - 2)

            emit_mm(2 * NT - 2)
            emit_mm(2 * NT - 1, last_tile=True)

            # ---- wrap-up: fold into gstat[128,5] + sxx row, spread over
            # ACT (PSUM copies) / DVE (diag extracts) ----
            gstat = stats.tile([128, 5], f32, name="gstat")
            scratch = stats.tile([128, 128], f32, name="scratch")
            # sx fold (DVE; ready before the matmuls finish)
            nc.vector.tensor_reduce(gstat[:, 0:1], sx_cols[:], axis=AX.X,
                                    op=ALU.add)
            # A, B2 from the ones columns (ACT sits next to PSUM)
            nc.scalar.activation(gstat[:, 1:2], P1[:, 128:129], AF.Copy)
            nc.scalar.activation(gstat[:, 2:3], P2[:, 128:129], AF.Copy)
            # Ta, Tb from the diagonals (DVE)
            nc.vector.tensor_tensor(scratch[:], P1[:, 0:128], identf[:],
                                    ALU.mult)
            nc.vector.tensor_reduce(gstat[:, 3:4], scratch[:], axis=AX.X,
                                    op=ALU.add)
            nc.vector.tensor_tensor(scratch[:], P2[:, 0:128], identf[:],
                                    ALU.mult)
            nc.vector.tensor_reduce(gstat[:, 4:5], scratch[:], axis=AX.X,
                                    op=ALU.add)
            # sxx: P3 row straight out via its own tiny DMA (no PE fold)
            srow = stats.tile([1, 128], f32, name="srow")
            nc.scalar.activation(srow[:], P3[0:1, 0:128], AF.Copy)

            nc.sync.dma_start(out=rows_out[:], in_=srow[:])
            nc.sync.dma_start(out=stats_out[:], in_=gstat[:])

    return nc


MODE = "host"


def get_nc(use_collective=True, stats_output=True):
    key = ("nc_v10",)
    if key not in _CACHE:
        nc = _build_nc()
        if not nc.is_finalized():
            nc.finalize()
        _CACHE[key] = nc
    return _CACHE[key]


def make_in_maps(x, p_mu, p_logvar):
    x = np.ascontiguousarray(np.asarray(x, dtype=np.float32))
    p_mu = np.ascontiguousarray(np.asarray(p_mu, dtype=np.float32))
    p_logvar = np.ascontiguousarray(np.asarray(p_logvar, dtype=np.float32))
    in_maps = []
    for c in range(NCORES):
        in_maps.append({
            "x": np.ascontiguousarray(
                x[c * BPC:(c + 1) * BPC].reshape(BPC, D, HW)),
            "p_mu": np.ascontiguousarray(p_mu[c * ROWS:(c + 1) * ROWS]),
            "p_logvar": np.ascontiguousarray(
                p_logvar[c * ROWS:(c + 1) * ROWS]),
        })
    return in_maps


def kernel(x, p_mu, p_logvar):
    from concourse.bass_utils import run_bass_kernel_spmd

    in_maps = make_in_maps(x, p_mu, p_logvar)
    nc = get_nc()
    res = run_bass_kernel_spmd(nc, in_maps, list(range(NCORES)))
    s = np.zeros((128, 5), dtype=np.float64)
    sxx = np.zeros(128, dtype=np.float64)
    for c in range(NCORES):
        s += np.asarray(res.results[c]["stats"], dtype=np.float64)
        sxx += np.asarray(res.results[c]["rows"],
                          dtype=np.float64).reshape(128)
    sx, A, B2p, Ta, Tb = (s[:, k] for k in range(5))
    T = Ta.sum() - 2.0 * Tb.sum()
    loss = -0.5 / N * (T - sxx.dot(A) / N + sx.dot(2.0 * B2p) / N)
    return np.asarray(loss, dtype=np.float32).reshape(())


# revision 22
# speedup vs baseline: 1.2957x; 1.2170x over previous
"""CLUB loss kernel for Trainium2, 8 NeuronCores (SPMD data-parallel).

Math: with flat_x (N,d), iv = exp(-p_logvar):
  positive_i = -0.5 * sum_d (x_i - mu_i)^2 * iv_i
  negative_i = -0.5 * sum_d iv_i * (ex2 - 2 mu_i ex + mu_i^2)
  loss = mean_i(positive_i - negative_i)
Decomposed into global sums (single pass over data):
  sx[d], sxx[d], A[d]=sum iv, B2[d]=sum iv*mu, Ta=sum iv*x^2, Tb=sum iv*mu*x
  loss = -0.5/N * [(Ta - 2 Tb) - dot(sxx,A)/N + dot(sx,2*B2)/N]

v2 design (vs v1 which was DVE/GPSIMD-bound and DMA-window-bound):
 - All DRAM loads are fully contiguous (8KB per partition): mu/lv land
   "i-major permuted" (partition p holds rows 16p..16p+15 of its 2048-row
   tile). All reductions are permutation-invariant over rows, so any
   row->partition assignment works as long as the x side matches.
 - x (d-major natural) is cast to fp16 and PE-transposed per 128-col group
   with column stride 16, which reproduces exactly the same row permutation
   (partition k of transpose block r = row 16k+r of the tile).
 - The coupled sums run on the PE as per-block [128x128] matmuls
   accumulated in PSUM over all 64 blocks:
     M1: lhsT=iv_blk,  rhs=[xsqT_blk | ones] -> diag = Ta partials, col128 = A
     M2: lhsT=j_blk,   rhs=[xT_blk   | ones] -> diag = Tb partials, col128 = B2
   fp16 operands (1 cyc/col); fp32 PSUM accumulation. fp16 keeps the
   cancellation-amplified error at ~1e-3 (measured vs reference), bf16 would
   be ~2.4e-3 and fp32 matmul is 4x slower.
 - sxx via GPSIMD partition-reduce (axis=C) of xsqT; sx via DVE free-dim
   reduce of the natural d-major x (engine balancing).
Device emits a per-core (128,6) stats block; host does the O(d) combine.
"""

import numpy as np

B, D, H, W = 16, 128, 64, 64
N = B * H * W            # 65536
NCORES = 8
BPC = B // NCORES        # 2 batches per core
HW = H * W               # 4096
ROWS = BPC * HW          # 8192 rows per core
TILE = 2048              # rows per tile
NT = ROWS // TILE        # 4 tiles per core
BLK = TILE // 128        # 16 transpose blocks per tile
NBLK = ROWS // 128       # 64 blocks per core

_CACHE = {}


def _build_nc(stats_output=True):
    import concourse.bass as bass
    import concourse.bacc as bacc
    import concourse.mybir as mybir
    from concourse import masks
    from concourse.tile import TileContext

    f32 = mybir.dt.float32
    f16 = mybir.dt.float16
    ALU = mybir.AluOpType
    AF = mybir.ActivationFunctionType
    AX = mybir.AxisListType

    nc = bacc.Bacc(num_devices=NCORES)
    x_in = nc.dram_tensor("x", [BPC, D, HW], f32, kind="ExternalInput")
    mu_in = nc.dram_tensor("p_mu", [ROWS, D], f32, kind="ExternalInput")
    lv_in = nc.dram_tensor("p_logvar", [ROWS, D], f32, kind="ExternalInput")
    stats_out = nc.dram_tensor("stats", [128, 6], f32, kind="ExternalOutput")

    with TileContext(nc) as tc:
        with (
            tc.tile_pool(name="const", bufs=1) as constp,
            tc.tile_pool(name="slabs", bufs=4) as slabs,
            tc.tile_pool(name="big", bufs=1) as big,
            tc.tile_pool(name="work", bufs=2) as work,
            tc.tile_pool(name="stats", bufs=1) as stats,
            tc.tile_pool(name="ps", bufs=2, space="PSUM") as psp,
            tc.tile_pool(name="psacc", bufs=1, space="PSUM") as psacc,
        ):
            # issue every input DMA first: the triggers have no deps, and
            # the 16 DMA engines stream ~12.6MB for ~32us — the earlier
            # they start, the earlier the whole pipeline finishes
            slabs_xml = []
            for t in range(NT):
                b, h = t // 2, t % 2
                x_t = slabs.tile([128, TILE], f32, tag="x_t", name="x_t")
                mu_t = slabs.tile([128, TILE], f32, tag="mu_t", name="mu_t")
                lv_t = slabs.tile([128, TILE], f32, tag="lv_t", name="lv_t")
                r0 = t * TILE
                nc.sync.dma_start(out=x_t[:],
                                  in_=x_in[b, :, h * TILE:(h + 1) * TILE])
                nc.sync.dma_start(
                    out=mu_t[:],
                    in_=mu_in[r0:r0 + TILE, :].rearrange(
                        "(p r) d -> p (r d)", p=128))
                nc.sync.dma_start(
                    out=lv_t[:],
                    in_=lv_in[r0:r0 + TILE, :].rearrange(
                        "(p r) d -> p (r d)", p=128))
                slabs_xml.append((x_t, mu_t, lv_t))

            ident16 = constp.tile([128, 128], f16, name="ident16")
            masks.make_identity(nc, ident16[:])
            identf = constp.tile([128, 128], f32, name="identf")
            masks.make_identity(nc, identf[:])
            onecell = constp.tile([1, 1], f32, name="onecell")
            nc.vector.memset(onecell[:], 1.0)
            ones_col = constp.tile([128, 1], f16, name="ones_col")
            nc.vector.memset(ones_col[:], 1.0)

            # persistent transposed-x layouts with a ones column every 129
            xTs = big.tile([128, NBLK * 129], f16, name="xTs")
            xsqTs = big.tile([128, NBLK * 129], f16, name="xsqTs")
            xTs_v = xTs[:].rearrange("p (n c) -> p n c", c=129)
            xsqTs_v = xsqTs[:].rearrange("p (n c) -> p n c", c=129)
            nc.vector.memset(xTs_v[:, :, 128:129], 1.0)
            nc.vector.memset(xsqTs_v[:, :, 128:129], 1.0)

            # PSUM accumulators for the coupled matmul streams
            P1 = psacc.tile([128, 129], f32, name="P1")
            P2 = psacc.tile([128, 129], f32, name="P2")
            P3 = psacc.tile([1, 129], f32, name="P3")

            sx_cols = stats.tile([128, 2 * NT], f32, name="sx_cols")

            # per-tile state kept across the 1-half-tile software pipeline lag
            ivs, jjs, psxs = {}, {}, {}

            def emit_mm(u):
                """M1/M2/M3 matmuls for half-tile u (blocks 8u..8u+8)."""
                t = u // 2
                iv, jj = ivs[t], jjs[t]
                for k in range(8):
                    blk = u * 8 + k          # global block in 0..63
                    r = (blk % BLK)          # block index within tile
                    first = blk == 0
                    last = blk == NBLK - 1
                    nc.tensor.matmul(
                        P1[:], iv[:, r * 128:(r + 1) * 128],
                        xsqTs_v[:, blk, :], start=first, stop=last,
                        skip_group_check=True)
                    nc.tensor.matmul(
                        P2[:], jj[:, r * 128:(r + 1) * 128],
                        xTs_v[:, blk, :], start=first, stop=last,
                        skip_group_check=True)
                    nc.tensor.matmul(
                        P3[:], ones_col[:],
                        xsqTs_v[:, blk, :], start=first, stop=last,
                        skip_group_check=True)

            for t in range(NT):
                x_t, mu_t, lv_t = slabs_xml[t]
                iv = work.tile([128, TILE], f16, tag="iv", name="iv", bufs=4)
                jj = work.tile([128, TILE], f16, tag="jj", name="jj", bufs=4)
                xb = work.tile([128, TILE], f16, tag="xb", name="xb", bufs=4)
                ivs[t], jjs[t] = iv, jj
                # ACT: xb = fp16(x)  (DVE CAST is ~4.4 cyc/elem; the
                # transposes read stride-16 columns so they need all of xb)
                nc.scalar.activation(xb[:], x_t[:], AF.Copy)
                QT = TILE // 4
                for q in range(4):
                    sl = slice(q * QT, (q + 1) * QT)
                    # ACT: iv = exp(-lv)  (f32 -> f16).  ACT carries ONLY
                    # exp+xb so the last tile's exp isn't queued behind
                    # x-side work; quarters let jj chase the exp.
                    nc.scalar.activation(iv[:, sl], lv_t[:, sl], AF.Exp,
                                         bias=0.0, scale=-1.0)
                    # GPSIMD: j = iv * mu (mixed f16*f32 -> f16; DVE's
                    # mixed path is a 131 cyc/elem microcode disaster)
                    nc.gpsimd.tensor_tensor(jj[:, sl], iv[:, sl],
                                            mu_t[:, sl], ALU.mult)
                HT = TILE // 2
                for hh in range(2):
                    sl = slice(hh * HT, (hh + 1) * HT)
                    # DVE: sx partial (d-major, f32)
                    nc.vector.tensor_reduce(sx_cols[:, 2 * t + hh:
                                                    2 * t + hh + 1],
                                            x_t[:, sl], axis=AX.X, op=ALU.add)

                # stride-16 column view of xb: xb_g[:, k, r] = xb[:, 16k+r]
                xb_g = xb[:].rearrange("p (k s) -> p k s", s=16)
                for hh in range(2):
                    u = t * 2 + hh
                    psx = psp.tile([128, 1024], f16, tag="psx", name="psx",
                                   bufs=4)
                    psxs[u] = psx
                    for k in range(8):
                        r = hh * 8 + k
                        nc.tensor.transpose(psx[:, k * 128:(k + 1) * 128],
                                            xb_g[:, :, r], ident16[:])
                    # DVE: plain copy into the ones-strided layout
                    blk0 = u * 8
                    nc.vector.tensor_copy(
                        xTs_v[:, blk0:blk0 + 8, 0:128], psx[:])
                    # DVE: square as xT(SBUF) * psx(PSUM) — keeps squares
                    # off ACT (so exp never queues behind them) and uses
                    # one read port per space (dual-PSUM TT is illegal)
                    nc.vector.tensor_tensor(
                        xsqTs_v[:, blk0:blk0 + 8, 0:128],
                        xTs_v[:, blk0:blk0 + 8, 0:128], psx[:], ALU.mult)
                    if u >= 2:
                        emit_mm(u - 2)

            for u in (2 * NT - 2, 2 * NT - 1):
                emit_mm(u)

            # ---- wrap-up: fold everything into g[128, 6] ----
            g = stats.tile([128, 6], f32, name="g")
            scratch = stats.tile([128, 128], f32, name="scratch")
            # sx
            nc.vector.tensor_reduce(g[:, 0:1], sx_cols[:], axis=AX.X,
                                    op=ALU.add)
            # A, B2 from the ones columns
            nc.vector.tensor_copy(g[:, 1:2], P1[:, 128:129])
            nc.vector.tensor_copy(g[:, 2:3], P2[:, 128:129])
            # Ta, Tb from the diagonals
            nc.vector.tensor_tensor(scratch[:], P1[:, 0:128], identf[:],
                                    ALU.mult)
            nc.vector.tensor_reduce(g[:, 3:4], scratch[:], axis=AX.X,
                                    op=ALU.add)
            nc.vector.tensor_tensor(scratch[:], P2[:, 0:128], identf[:],
                                    ALU.mult)
            nc.vector.tensor_reduce(g[:, 4:5], scratch[:], axis=AX.X,
                                    op=ALU.add)
            # sxx: P3 row [1,128] -> SBUF, then to a column via the PE
            srow = stats.tile([1, 128], f32, name="srow")
            nc.vector.tensor_copy(srow[:], P3[0:1, 0:128])
            psC = psacc.tile([128, 1], f32, name="psC")
            nc.tensor.matmul(psC[:], srow[:], onecell[:], start=True,
                             stop=True)
            nc.vector.tensor_copy(g[:, 5:6], psC[:])

            nc.sync.dma_start(out=stats_out[:], in_=g[:])

    return nc


MODE = "host"


def get_nc(use_collective=True, stats_output=True):
    key = ("nc_v2",)
    if key not in _CACHE:
        nc = _build_nc()
        if not nc.is_finalized():
            nc.finalize()
        _CACHE[key] = nc
    return _CACHE[key]


def make_in_maps(x, p_mu, p_logvar):
    x = np.ascontiguousarray(np.asarray(x, dtype=np.float32))
    p_mu = np.ascontiguousarray(np.asarray(p_mu, dtype=np.float32))
    p_logvar = np.ascontiguousarray(np.asarray(p_logvar, dtype=np.float32))
    in_maps = []
    for c in range(NCORES):
        in_maps.append({
            "x": np.ascontiguousarray(
                x[c * BPC:(c + 1) * BPC].reshape(BPC, D, HW)),
            "p_mu": np.ascontiguousarray(p_mu[c * ROWS:(c + 1) * ROWS]),
            "p_logvar": np.ascontiguousarray(
                p_logvar[c * ROWS:(c + 1) * ROWS]),
        })
    return in_maps


def kernel(x, p_mu, p_logvar):
    from concourse.bass_utils import run_bass_kernel_spmd

    in_maps = make_in_maps(x, p_mu, p_logvar)
    nc = get_nc()
    res = run_bass_kernel_spmd(nc, in_maps, list(range(NCORES)))
    s = np.zeros((128, 6), dtype=np.float64)
    for c in range(NCORES):
        s += np.asarray(res.results[c]["stats"], dtype=np.float64)
    sx, A, B2p, Ta, Tb, sxx = (s[:, k] for k in range(6))
    T = Ta.sum() - 2.0 * Tb.sum()
    loss = -0.5 / N * (T - sxx.dot(A) / N + sx.dot(2.0 * B2p) / N)
    return np.asarray(loss, dtype=np.float32).reshape(())
